# revision 63
# baseline (speedup 1.0000x reference)
"""Multi-head attention (B=2, N=2048, C=1024, H=16, D=64) on 8 TRN2 NeuronCores.

Sharding: tensor-parallel over heads (2 heads/core), both batches on every
core; output projection row-parallel over the core's 128 attention-output
channels; host sums the 8 partial y tensors and adds the bias.

Per-core schedule (static, engine-queue aware):
  - Act engine runs ONLY the 128 softmax-exp activations ([128,1024] each).
  - PE: bf16 qkv projection, scores as fp8 DoubleRow with dithered dual
    quantization (both DR slots carry differently-scaled fp8 copies of q/k;
    their products average, halving quantization noise at zero PE cost),
    bf16 AV with the packed [V|ones] stationary trick producing softmax
    denominators for free, bf16 output projection.
  - DVE: all PSUM evacuations, reciprocal+multiply normalization (deferred
    into the next attention block to avoid head-of-line blocking).
  - Engine queues execute in order, so qkv/transpose/proj work is emitted as
    fine-grained filler quanta INSIDE the attention j-loops.
"""

import sys

sys.path.insert(0, "/opt/trn_rl_repo")

import numpy as np
import ml_dtypes

import concourse.mybir as mybir
import concourse.tile as tile
from concourse import bacc
from concourse.bass_utils import run_bass_kernel_spmd
from concourse.masks import make_identity

F32 = mybir.dt.float32
BF16 = mybir.dt.bfloat16
FP8 = mybir.dt.float8e4
AF = mybir.ActivationFunctionType
DR = mybir.MatmulPerfMode.DoubleRow

BF = ml_dtypes.bfloat16

B = 2
N = 2048
C = 1024
H = 16
D = 64
NCORES = 8
HPC = H // NCORES          # heads per core = 2
CT = C // 128              # contraction tiles = 8
NT = N // 128              # m tiles = 16
NCH = N // 512             # 512-wide n chunks = 4
SCALE = float(D) ** -0.5

SCORES_FP8 = True          # dithered fp8 DoubleRow scores vs plain bf16
DITHER_C = 1.3
# (j, ci) attention slots whose exp runs on the DVE via the Schraudolph
# bf16-bits trick (round(x*128/ln2 + 16248.6) as uint16 == bf16(exp(x)) to
# ~3%), relieving the Act engine which otherwise binds the schedule
SCH_SLOTS = {(3, 0)}
SCH_B = 127.0 * 128.0 - 0.0579 * 128.0


def _build():
    nc = bacc.Bacc("TRN2")
    xTr = nc.dram_tensor("xTr", [B, 128, CT, N], BF16, kind="ExternalInput")
    wT = nc.dram_tensor("wT", [128, CT, 3, 128], BF16, kind="ExternalInput")
    wpT = nc.dram_tensor("wpT", [128, C], BF16, kind="ExternalInput")
    y = nc.dram_tensor("y", [B, N, C], BF16, kind="ExternalOutput")

    with tile.TileContext(nc) as tc:
        with tc.tile_pool(name="consts", bufs=1) as consts, \
             tc.tile_pool(name="xt", bufs=2) as xt_pool, \
             tc.tile_pool(name="qk", bufs=2) as qk_pool, \
             tc.tile_pool(name="vt", bufs=2) as vt_pool, \
             tc.tile_pool(name="vo", bufs=2) as vo_pool, \
             tc.tile_pool(name="et", bufs=4) as et_pool, \
             tc.tile_pool(name="oc", bufs=2) as oc_pool, \
             tc.tile_pool(name="rec", bufs=2) as rec_pool, \
             tc.tile_pool(name="yo", bufs=8) as yo_pool, \
             tc.tile_pool(name="ps_s", bufs=2, space="PSUM") as ps_s, \
             tc.tile_pool(name="ps_av", bufs=2, space="PSUM") as ps_av, \
             tc.tile_pool(name="ps_mm", bufs=2, space="PSUM") as ps_mm:

            wt_sb = consts.tile([128, CT, 3, 128], BF16)
            wp_sb = consts.tile([128, C], BF16)
            ident_bf = consts.tile([128, 128], BF16)
            # weights ride the Activation HWDGE queue so descriptor gen
            # overlaps the x loads on the SP queue; wT is issued first so its
            # transfer wins the (serial) DMA pipe — every matmul needs it.
            nc.scalar.dma_start(out=wt_sb, in_=wT[:, :, :, :])
            make_identity(nc, ident_bf[:, :])
            # scratch for PE warm-up matmuls: no DMA dependency, so the PE
            # p-state ramps to full clock before the real qkv work arrives
            warm_bf = consts.tile([128, 512], BF16)
            nc.gpsimd.memset(warm_bf, 1.0)

            st = {b: {} for b in range(B)}

            def load_x(b):
                # split by n-chunk: the first qkv psum (which contracts over
                # ALL ct tiles of one n-chunk) unblocks after 1/4 of the load
                t = xt_pool.tile([128, CT, N], BF16, tag="xt", name=f"xt{b}")
                for nch in range(NCH):
                    nc.sync.dma_start(
                        out=t[:, :, nch * 512:(nch + 1) * 512],
                        in_=xTr[b, :, :, nch * 512:(nch + 1) * 512])
                st[b]["xt"] = t

            def alloc_batch(b):
                if SCORES_FP8:
                    st[b]["qf"] = qk_pool.tile([128, 2, N], FP8, tag="qf",
                                               name=f"qf{b}")
                    st[b]["kf"] = qk_pool.tile([128, 2, N], FP8, tag="kf",
                                               name=f"kf{b}")
                else:
                    st[b]["qf"] = qk_pool.tile([128, N], BF16, tag="qf",
                                               name=f"qf{b}")
                    st[b]["kf"] = qk_pool.tile([128, N], BF16, tag="kf",
                                               name=f"kf{b}")
                st[b]["vt"] = vt_pool.tile([128, N], BF16, tag="vt", name=f"vt{b}")
                st[b]["vo"] = vo_pool.tile([128, NT, 192], BF16, tag="vo",
                                           name=f"vo{b}")
                st[b]["oc"] = oc_pool.tile([128, N], BF16, tag="oc", name=f"oc{b}")
                nc.vector.memset(st[b]["vo"][:, :, 64:128], 1.0)

            prog = set()

            def qkv_part(b, parts):
                """Emit qkv projection for (si, nch) pairs; yield every 2 mms."""
                xt = st[b]["xt"]
                qf, kf, vt = st[b]["qf"], st[b]["kf"], st[b]["vt"]
                for si, nch in parts:
                    csl = slice(nch * 512, (nch + 1) * 512)
                    ps = ps_mm.tile([128, 512], F32, tag="mm",
                                    name=f"qkv{b}_{nch}_{si}")
                    for ct in range(CT):
                        nc.tensor.matmul(
                            ps[:, :],
                            wt_sb[:, ct, si, :],
                            xt[:, ct, csl],
                            start=(ct == 0), stop=(ct == CT - 1),
                        )
                        if ct % 2 == 1:
                            yield
                    if si == 2:
                        nc.vector.tensor_copy(vt[:, csl], ps[:, :])
                    elif SCORES_FP8:
                        dst = qf if si == 0 else kf
                        ca = DITHER_C if si == 0 else 1.0 / DITHER_C
                        # the very first q/k evacs gate the first exp; put one
                        # dither copy on the (still idle) Act engine there
                        slot1 = (nc.scalar.mul if (b == 0 and nch <= 1)
                                 else lambda o, i, m:
                                 nc.vector.tensor_scalar_mul(o, i, m))
                        nc.vector.tensor_scalar_mul(dst[:, 0, csl], ps[:, :], ca)
                        slot1(dst[:, 1, csl], ps[:, :], 1.0 / ca)
                    else:
                        dst = qf if si == 0 else kf
                        nc.vector.tensor_copy(dst[:, csl], ps[:, :])
                    prog.add(("qkv"[si], b, nch))
                    yield

            def tpvo_part(b, mts):
                """PE-transpose V m-tiles into [V_h0 | ones | V_h1] slabs."""
                vt, vo = st[b]["vt"], st[b]["vo"]
                for mt in mts:
                    tp = ps_mm.tile([128, 128], BF16, tag="mm", name=f"tp{b}_{mt}")
                    nc.tensor.transpose(
                        tp[:, :], vt[:, mt * 128:(mt + 1) * 128], ident_bf[:, :])
                    nc.vector.tensor_copy(vo[:, mt, 0:64], tp[:, 0:64])
                    nc.vector.tensor_copy(vo[:, mt, 128:192], tp[:, 64:128])
                    prog.add(("vo", b, mt))
                    yield

            def proj_part(b, nts, wide=False):
                """Output projection + evac + store for the given n-tiles.

                wide=True uses the (by then idle) ps_s pool with 1024-wide
                psum tiles — fewer rotation stalls for the epilogue.
                """
                oc = st[b]["oc"]
                for nt in nts:
                    if wide:
                        # epilogue: ps_s/ps_av are idle — rotate over both,
                        # and alternate evac between DVE and the idle Act
                        pp = ps_s.tile([128, 1024], F32, tag="s",
                                       name=f"ppw{b}_{nt}")
                        for och in range(2):
                            nc.tensor.matmul(
                                pp[:, och * 512:(och + 1) * 512],
                                oc[:, nt * 128:(nt + 1) * 128],
                                wp_sb[:, och * 512:(och + 1) * 512],
                                start=True, stop=True,
                            )
                        ysb = yo_pool.tile([128, 1024], BF16, tag="yow",
                                           name=f"ysbw{b}_{nt}")
                        cp = nc.vector.tensor_copy if nt % 2 else nc.scalar.copy
                        cp(ysb[:, :], pp[:, :])
                        eng = nc.sync if nt % 2 else nc.scalar
                        eng.dma_start(
                            out=y[b, nt * 128:(nt + 1) * 128, :],
                            in_=ysb[:, :],
                        )
                        yield
                        continue
                    for och in range(2):
                        pp = ps_mm.tile([128, 512], F32, tag="mm",
                                        name=f"pp{b}_{nt}_{och}")
                        nc.tensor.matmul(
                            pp[:, :],
                            oc[:, nt * 128:(nt + 1) * 128],
                            wp_sb[:, och * 512:(och + 1) * 512],
                            start=True, stop=True,
                        )
                        ysb = yo_pool.tile([128, 512], BF16, tag="yo",
                                           name=f"ysb{b}_{nt}_{och}")
                        nc.vector.tensor_copy(ysb[:, :], pp[:, :])
                        nc.sync.dma_start(
                            out=y[b, nt * 128:(nt + 1) * 128,
                                  och * 512:(och + 1) * 512],
                            in_=ysb[:, :],
                        )
                        yield

            def take(fillers, n):
                done = 0
                while done < n and fillers:
                    try:
                        next(fillers[0])
                        done += 1
                    except StopIteration:
                        fillers.pop(0)
                return done

            def need(fillers, key):
                while key not in prog:
                    if not take(fillers, 1):
                        raise RuntimeError(f"unreachable dep {key}")

            def attn_block(b, hl, qp, fillers, quota, post, sch=frozenset()):
                """One (batch, head, chunk-pair) attention block.

                `post` (the previous block's deferred normalizations) is
                emitted at slot j==0 so the DVE never parks on an unfinished
                AV accumulation.  Data dependencies on filler-produced tiles
                are enforced by need() pulls (emission order == dataflow).
                """
                hs = hl * 64
                qf, kf, vo, oc = (st[b][k] for k in ("qf", "kf", "vo", "oc"))
                chunks = (2 * qp, 2 * qp + 1)
                avs = [ps_av.tile([128, 512], F32, tag="av",
                                  name=f"av{b}_{hl}_{qp}_{ci}")
                       for ci in range(2)]
                prev = None

                def emit_av(j, ets):
                    for ci, et_t in ets:
                        for half, mt in ((0, 2 * j), (1, 2 * j + 1)):
                            nc.tensor.matmul(
                                avs[ci][:, :],
                                vo[:, mt, hs:hs + 128],
                                et_t[:, half * 512:(half + 1) * 512],
                                start=(mt == 0), stop=(mt == NT - 1),
                            )

                for j in range(NT // 2):
                    need(fillers, ("k", b, (2 * j + 1) // 4))
                    if prev is not None:
                        need(fillers, ("vo", b, 2 * prev[0] + 1))
                        emit_av(*prev)
                        prev = None
                    cur = []
                    for ci, ch in enumerate(chunks):
                        need(fillers, ("q", b, ch))
                        s = ps_s.tile([128, 1024], F32, tag="s",
                                      name=f"s{b}_{hl}_{qp}_{j}_{ci}")
                        for half, mt in ((0, 2 * j), (1, 2 * j + 1)):
                            osl = slice(half * 512, (half + 1) * 512)
                            if SCORES_FP8:
                                nc.tensor.matmul(
                                    s[:, osl],
                                    kf[hs:hs + 64, :, mt * 128:(mt + 1) * 128],
                                    qf[hs:hs + 64, :, ch * 512:(ch + 1) * 512],
                                    start=True, stop=True, perf_mode=DR,
                                )
                            else:
                                nc.tensor.matmul(
                                    s[:, osl],
                                    kf[hs:hs + 64, mt * 128:(mt + 1) * 128],
                                    qf[hs:hs + 64, ch * 512:(ch + 1) * 512],
                                    start=True, stop=True,
                                )
                        et_t = et_pool.tile([128, 1024], BF16, tag="et",
                                            name=f"et{b}_{hl}_{qp}_{j}_{ci}")
                        sc = SCALE / 2.0 if SCORES_FP8 else SCALE
                        if (j, ci) in sch:
                            nc.vector.tensor_scalar(
                                et_t.bitcast(mybir.dt.uint16)[:, :], s[:, :],
                                sc * 128.0 / float(np.log(2.0)), SCH_B,
                                mybir.AluOpType.mult, mybir.AluOpType.add)
                        else:
                            nc.scalar.activation(
                                out=et_t[:, :], in_=s[:, :], func=AF.Exp,
                                scale=sc)
                        cur.append((ci, et_t))
                        take(fillers, 1)
                    if j == 0:
                        for fn in post:
                            fn()
                        post = []
                    take(fillers, max(0, quota - 2))
                    prev = (j, cur)
                need(fillers, ("vo", b, 2 * prev[0] + 1))
                emit_av(*prev)

                osl = slice(0, 64) if hl == 0 else slice(64, 128)
                dsl = slice(64, 128) if hl == 0 else slice(0, 64)

                def norm(ci, ch):
                    def fn():
                        rec = rec_pool.tile([128, 512], F32, tag="rec",
                                            name=f"rec{b}_{hl}_{qp}_{ci}")
                        nc.vector.reciprocal(rec[dsl, :], avs[ci][dsl, :])
                        nc.vector.tensor_mul(
                            oc[hs:hs + 64, ch * 512:(ch + 1) * 512],
                            avs[ci][osl, :],
                            rec[dsl, :],
                        )
                        prog.add(("oc", b, hl, ch))
                    return fn

                return [norm(ci, ch) for ci, ch in enumerate(chunks)]

            # ---------------- static schedule ----------------
            load_x(0)
            load_x(1)
            nc.scalar.dma_start(out=wp_sb, in_=wpT[:, :])
            alloc_batch(0)
            alloc_batch(1)

            # PE warm-up: ~10 dummy matmuls on SBUF-resident data ramp the
            # tensor engine to full clock while the x/w DMAs stream in
            wps = ps_s.tile([128, 1024], F32, tag="s", name="warmup")
            for _ in range(7):
                nc.tensor.matmul(wps[:, 0:512], ident_bf[:, :], warm_bf[:, :],
                                 start=True, stop=True)

            # filler list in strict first-need order: batch-0's remaining
            # producers (its own attention consumes them first), then
            # batch-1's stream, then late q chunks; proj streams are appended
            # once their oc chunks exist. need() is the safety net.
            fillers = [
                qkv_part(0, [(0, 1)]),
                qkv_part(0, [(1, 1), (2, 1)]),
                tpvo_part(0, range(4, 8)),
                qkv_part(0, [(1, 2), (2, 2)]),
                tpvo_part(0, range(8, 12)),
                qkv_part(0, [(1, 3), (2, 3)]),
                tpvo_part(0, range(12, 16)),
                qkv_part(0, [(0, 2), (0, 3)]),
                qkv_part(1, [(0, 0), (1, 0), (2, 0), (0, 1)]),
                tpvo_part(1, range(0, 4)),
                qkv_part(1, [(1, 1), (2, 1)]),
                tpvo_part(1, range(4, 8)),
                qkv_part(1, [(1, 2), (2, 2)]),
                tpvo_part(1, range(8, 12)),
                qkv_part(1, [(1, 3), (2, 3)]),
                tpvo_part(1, range(12, 16)),
                qkv_part(1, [(0, 2), (0, 3)]),
            ]

            # prologue: exactly what the first exp of block (b0,h0,qp0) needs
            pro = [qkv_part(0, [(0, 0), (1, 0), (2, 0)]),
                   tpvo_part(0, range(0, 4))]
            while take(pro, 64):
                pass

            post = []
            blocks = [(0, 0, 0), (0, 1, 0), (0, 0, 1), (0, 1, 1),
                      (1, 0, 0), (1, 1, 0), (1, 0, 1), (1, 1, 1)]
            for bi, (b, hl, qp) in enumerate(blocks):
                if bi == 2:
                    fillers.append(proj_part(0, range(0, 8)))
                if bi == 4:
                    fillers.append(proj_part(0, range(8, NT)))
                if bi == 6:
                    fillers.append(proj_part(1, range(0, 8)))
                post = attn_block(b, hl, qp, fillers, 2, post)
            for fn in post:
                fn()
            while take(fillers, 64):
                pass
            for _ in proj_part(1, range(8, NT), wide=True):
                pass
    nc.finalize()
    return nc


_NC = None


def _get_nc():
    global _NC
    if _NC is None:
        _NC = _build()
    return _NC


def _prep_shared(x):
    # x [B, N, C] -> xTr [B, 128, CT, N] bf16 (c = ct*128 + p)
    xT = x.transpose(0, 2, 1).reshape(B, CT, 128, N).transpose(0, 2, 1, 3)
    return np.ascontiguousarray(xT).astype(BF)


def kernel(x, w_qkv, w_proj, b_proj):
    x = np.asarray(x, dtype=np.float32)
    w_qkv = np.asarray(w_qkv, dtype=np.float32)
    w_proj = np.asarray(w_proj, dtype=np.float32)
    b_proj = np.asarray(b_proj, dtype=np.float32)

    xTr = _prep_shared(x)
    in_maps = []
    for core in range(NCORES):
        h0 = core * HPC
        rows = np.concatenate(
            [np.arange(h * D, (h + 1) * D) for h in range(h0, h0 + HPC)])
        wsel = np.concatenate(
            [w_qkv[rows, :], w_qkv[C + rows, :], w_qkv[2 * C + rows, :]], axis=0)
        # [384, C] -> [C, 384] -> [CT, 128, 3, 128] -> [128, CT, 3, 128]
        wTa = wsel.T.reshape(CT, 128, 3, 128).transpose(1, 0, 2, 3)
        wTa = np.ascontiguousarray(wTa).astype(BF)
        cols = np.arange(h0 * D, (h0 + HPC) * D)
        wpT = np.ascontiguousarray(w_proj[:, cols].T).astype(BF)
        in_maps.append({"xTr": xTr, "wT": wTa, "wpT": wpT})

    nc = _get_nc()
    res = run_bass_kernel_spmd(nc, in_maps, core_ids=list(range(NCORES)))
    out = np.zeros((B, N, C), dtype=np.float32)
    for core in range(NCORES):
        out += res.results[core]["y"].astype(np.float32)
    out += b_proj
    return out
